# revision 1
# baseline (speedup 1.0000x reference)
# Trainium2 Bass kernel for nn_EARLIEST (adaptive-halting LSTM, B=128 T=4096
# V=128 H=256 C=10).
#
# Key observation: the model halts each batch sample at the first step t where
# u[b,t] < probs[b,t], with probs ~= 0.45 early on, so every sample halts
# within a few dozen steps (max 36 for the seed-0 inputs).  The returned
# output only needs logits at each sample's first halt step (or step T-1 for
# never-halted samples).  So the device kernel runs the LSTM scan for only
# T_EFF timesteps, emits pre-softmax logits and the halting dot-product for
# every (t, b), and the host applies the (exact) halting latch.  A numpy
# fallback continues the recurrence from the device's (h, c) state for any
# sample that has not halted by T_EFF (statistically never happens; the
# fallback keeps the kernel correct for arbitrary inputs).
#
# Sharding: data-parallel over batch, 16 samples per core, weights replicated.
# Layout on device is feature-major: h^T is [H=256, b=16] stored as two
# 128-partition k-tiles side by side, so LSTM gate math runs on full
# 128-partition tiles and the recurrent matmuls need no transposes.

import numpy as np
import ml_dtypes

import concourse.bass as bass
import concourse.mybir as mybir
from concourse.bass_utils import run_bass_kernel_spmd

B, T_FULL, V, H, C = 128, 4096, 128, 256, 10
EPS = 0.1
NCORES = 8
BL = B // NCORES  # 16 samples per core
T_EFF = 48
M_TILES = 8   # 4H/128
K2 = 2        # H/128
F32 = mybir.dt.float32
F16 = mybir.dt.float16

# gate order stays the native (i, f, g, o): with the all-tanh trick the
# only contiguity needed is [i,f,g] (first ACT chunk) and [o] (second).
GATE_PERM = np.arange(1024)


def _build(T):
    """Build the raw-bass single-core program (SPMD across 8 cores)."""
    nc = bass.Bass()

    d_Xt = nc.dram_tensor("Xt", [128, T * BL], F16, kind="ExternalInput")
    d_WkT = nc.dram_tensor("WkT", [128, 1024], F16, kind="ExternalInput")
    d_WrT = nc.dram_tensor("WrT", [128, 2048], F16, kind="ExternalInput")
    d_ident = nc.dram_tensor("ident", [128, 128], F16, kind="ExternalInput")
    d_blstm = nc.dram_tensor("blstm", [128, 8], F32, kind="ExternalInput")
    d_WoC = nc.dram_tensor("WoC", [128, 22], F16, kind="ExternalInput")
    d_bob = nc.dram_tensor("bob", [11, 1], F32, kind="ExternalInput")
    d_head = nc.dram_tensor("head", [11, T * BL], F32, kind="ExternalOutput")
    d_state_h = nc.dram_tensor("state_h", [128, 32], F16, kind="ExternalOutput")
    d_state_c = nc.dram_tensor("state_c", [128, 32], F32, kind="ExternalOutput")

    NH = T * BL
    HALF = NH // 2  # fp32 head matmul moving-operand limit is 512

    from contextlib import ExitStack
    ctx = ExitStack()
    sb_Xt = ctx.enter_context(nc.sbuf_tensor([128, T * BL], F16))
    sb_WkT = ctx.enter_context(nc.sbuf_tensor([128, 1024], F16))
    sb_WrT = ctx.enter_context(nc.sbuf_tensor([128, 2048], F16))
    sb_I = ctx.enter_context(nc.sbuf_tensor([128, 128], F16))
    sb_blstm = ctx.enter_context(nc.sbuf_tensor([128, 8], F32))
    sb_WoC = ctx.enter_context(nc.sbuf_tensor([128, 22], F16))
    sb_bob = ctx.enter_context(nc.sbuf_tensor([11, 1], F32))
    sb_XW = ctx.enter_context(nc.sbuf_tensor([128, T * 128], F16))
    sb_H = ctx.enter_context(nc.sbuf_tensor([128, (T + 1) * 32], F16))
    sb_C = ctx.enter_context(nc.sbuf_tensor([128, (T + 1) * 32], F32))
    sb_G = ctx.enter_context(nc.sbuf_tensor([128, 2 * 128], F32))
    sb_TC = ctx.enter_context(nc.sbuf_tensor([128, 2 * 32], F32))
    sb_S = ctx.enter_context(nc.sbuf_tensor([128, 2 * 32], F32))
    sb_U = ctx.enter_context(nc.sbuf_tensor([128, 32], F32))
    sb_Vt = ctx.enter_context(nc.sbuf_tensor([128, 32], F32))
    sb_head = ctx.enter_context(nc.sbuf_tensor([11, T * BL], F32))

    ps_pre = [ctx.enter_context(nc.psum_tensor(f"ps_pre{j}", [128, 512], F32))
              for j in range(2)]
    ps_z = [ctx.enter_context(nc.psum_tensor(f"ps_z{j}", [128, 512], F32))
            for j in range(2)]
    ps_hd = [ctx.enter_context(nc.psum_tensor(f"ps_hd{j}", [128, 512], F32))
             for j in range(2)]
    ps_s = ctx.enter_context(nc.psum_tensor("ps_s", [128, 512], F32))
    ps_warm = ctx.enter_context(nc.psum_tensor("ps_warm", [128, 512], F32))

    # one semaphore per input load: DMA completion order is not program order
    dma_xt = ctx.enter_context(nc.semaphore("dma_xt"))
    dma_wk = ctx.enter_context(nc.semaphore("dma_wk"))
    dma_wr = ctx.enter_context(nc.semaphore("dma_wr"))
    dma_id = ctx.enter_context(nc.semaphore("dma_id"))
    dma_bl = ctx.enter_context(nc.semaphore("dma_bl"))
    dma_wo = ctx.enter_context(nc.semaphore("dma_wo"))
    dma_bo = ctx.enter_context(nc.semaphore("dma_bo"))
    dma_out = ctx.enter_context(nc.semaphore("dma_out"))
    sem_pre = ctx.enter_context(nc.semaphore("sem_pre"))
    sem_precp = ctx.enter_context(nc.semaphore("sem_precp"))
    sem_h = ctx.enter_context(nc.semaphore("sem_h"))
    sem_cp = ctx.enter_context(nc.semaphore("sem_cp"))
    sem_act = ctx.enter_context(nc.semaphore("sem_act"))
    sem_pe = ctx.enter_context(nc.semaphore("sem_pe"))
    sem_hd = ctx.enter_context(nc.semaphore("sem_hd"))
    sem_hdcp = ctx.enter_context(nc.semaphore("sem_hdcp"))
    sem_uv = ctx.enter_context(nc.semaphore("sem_uv"))
    sem_cv = ctx.enter_context(nc.semaphore("sem_cv"))

    n_half = 2
    assert T % n_half == 0
    TH = T // n_half
    assert TH * BL == HALF

    with nc.Block() as block:

        @block.sync
        def _(sync):
            sync.dma_start(out=sb_Xt[:], in_=d_Xt[:]).then_inc(dma_xt, 16)
            sync.dma_start(out=sb_WkT[:], in_=d_WkT[:]).then_inc(dma_wk, 16)
            sync.dma_start(out=sb_WrT[:], in_=d_WrT[:]).then_inc(dma_wr, 16)
            sync.dma_start(out=sb_I[:], in_=d_ident[:]).then_inc(dma_id, 16)
            sync.dma_start(out=sb_blstm[:], in_=d_blstm[:]).then_inc(dma_bl, 16)
            sync.dma_start(out=sb_WoC[:], in_=d_WoC[:]).then_inc(dma_wo, 16)
            sync.dma_start(out=sb_bob[:], in_=d_bob[:]).then_inc(dma_bo, 16)
            sync.wait_ge(sem_hdcp, n_half)
            sync.dma_start(out=d_head[:], in_=sb_head[:]).then_inc(dma_out, 16)
            sync.wait_ge(sem_h, T + 1)
            sync.dma_start(out=d_state_h[:],
                           in_=sb_H[:, T * 32:(T + 1) * 32]).then_inc(dma_out, 16)
            sync.wait_ge(sem_cv, T)
            sync.dma_start(out=d_state_c[:],
                           in_=sb_C[:, T * 32:(T + 1) * 32]).then_inc(dma_out, 16)
            sync.wait_ge(dma_out, 48)

        @block.tensor
        def _(tensor):
            # ---- precompute XW = Wk^T X^T (feature-major, fp16) ----
            # half 0 runs up front; half 1 is interleaved into the scan.
            def pre_mm(idx):
                half, m = divmod(idx, M_TILES)
                if idx >= 2:
                    tensor.wait_ge(sem_precp, idx - 1)
                tensor.matmul(
                    ps_pre[idx % 2][:, 0:HALF],
                    sb_WkT[:, m * 128:(m + 1) * 128],
                    sb_Xt[:, half * HALF:(half + 1) * HALF],
                    start=True, stop=True,
                ).then_inc(sem_pre)

            tensor.wait_ge(dma_xt, 16)
            tensor.wait_ge(dma_wk, 16)
            for idx in range(M_TILES):
                pre_mm(idx)
            # ---- recurrent scan ----
            tensor.wait_ge(dma_wr, 16)
            tensor.wait_ge(dma_id, 16)
            tensor.wait_ge(dma_wo, 16)
            tensor.wait_ge(sem_precp, M_TILES)   # XW half 0 in SBUF
            h4 = sb_H[:].rearrange("p (t k b) -> p t k b", k=K2, b=BL)
            for t in range(T):
                if t == TH:
                    tensor.wait_ge(sem_precp, 2 * M_TILES)  # XW half 1
                if t >= 2:
                    # ps_z bank reuse: ACT consumed step t-2 gates
                    tensor.wait_ge(sem_act, 3 * (t - 2) + 2)
                # prefill z with XW[t] in one identity matmul BEFORE waiting
                # for h: it only depends on XW, so it runs during the tail
                # of step t-1 (and keeps PE a little warmer).
                tensor.matmul(ps_z[t % 2][:, 0:128], sb_I[:],
                              sb_XW[:, t * 128:(t + 1) * 128],
                              start=True, stop=True, skip_group_check=True)
                for _w in range(2):
                    tensor.matmul(ps_warm[:, 0:128], sb_I[:],
                                  sb_XW[:, t * 128:(t + 1) * 128],
                                  start=True, stop=True,
                                  skip_group_check=True)
                tensor.wait_ge(sem_h, t + 1)
                for m in range(M_TILES):
                    for k in range(K2):
                        mm = tensor.matmul(
                            ps_z[t % 2][:, m * BL:(m + 1) * BL],
                            sb_WrT[:, k * 1024 + m * 128:k * 1024 + (m + 1) * 128],
                            sb_H[:, t * 32 + k * BL:t * 32 + (k + 1) * BL],
                            start=False, stop=False, skip_group_check=True,
                        )
                    if m == 5:
                        mm.then_inc(sem_pe)  # i,f,g columns complete
                mm.then_inc(sem_pe)          # o columns complete
                if t < M_TILES:
                    pre_mm(M_TILES + t)
                if t == TH:
                    # head for h_1..h_TH — all its inputs exist by now, and
                    # PE is otherwise idle during the tail of each step
                    for k in range(K2):
                        tensor.matmul(
                            ps_hd[0][0:11, 0:HALF],
                            sb_WoC[:, k * 11:(k + 1) * 11],
                            h4[:, 1:1 + TH, k, :],
                            start=(k == 0), stop=(k == 1),
                        ).then_inc(sem_hd)
            # ---- head, second half (h_{TH+1}..h_T) ----
            tensor.wait_ge(sem_h, T + 1)
            for k in range(K2):
                tensor.matmul(
                    ps_hd[1][0:11, 0:HALF],
                    sb_WoC[:, k * 11:(k + 1) * 11],
                    h4[:, 1 + TH:1 + 2 * TH, k, :],
                    start=(k == 0), stop=(k == 1),
                ).then_inc(sem_hd)

        @block.vector
        def _(vector):
            vector.memset(sb_H[:, 0:32], 0.0)
            vector.memset(sb_C[:, 0:32], 0.0).then_inc(sem_h)
            # ---- precompute copies: psum + b_lstm -> XW (fp16) ----
            vector.wait_ge(dma_bl, 16)
            xw4 = sb_XW[:].rearrange("p (t m b) -> p t m b", m=M_TILES, b=BL)

            def pre_copy(idx):
                half, m = divmod(idx, M_TILES)
                vector.wait_ge(sem_pre, idx + 1)
                psrc = ps_pre[idx % 2][:, 0:HALF].rearrange(
                    "p (t b) -> p t b", b=BL)
                nc.vector.tensor_scalar_add(
                    xw4[:, half * TH:(half + 1) * TH, m, :], psrc,
                    sb_blstm[:, m:m + 1],
                ).then_inc(sem_precp)

            for idx in range(M_TILES):
                pre_copy(idx)
            # same-engine fence: v2(0) reads C written by memset above
            vector.drain()
            # ---- scan pointwise ----
            Alu = mybir.AluOpType
            for t in range(T):
                s = t % 2
                gs = sb_G[:, s * 128:(s + 1) * 128]
                ss = ps_s[:, s * 32:(s + 1) * 32]
                # all gates arrive as tanh (i,f,o weight cols pre-halved on
                # host): v2 = (tf+1)(.)c = 2f(.)c ; u2 = (ti+1)(.)tg ; S = 2c'
                if t >= 1:
                    vector.wait_ge(sem_cv, t)  # c(t) committed (same engine)
                vector.wait_ge(sem_act, 3 * t + 1)
                nc.vector.scalar_tensor_tensor(
                    sb_Vt[:], gs[:, 32:64], 1.0, sb_C[:, t * 32:(t + 1) * 32],
                    Alu.add, Alu.mult)
                nc.vector.scalar_tensor_tensor(
                    sb_U[:], gs[:, 0:32], 1.0, gs[:, 64:96],
                    Alu.add, Alu.mult).then_inc(sem_uv)
                vector.wait_ge(sem_uv, t + 1)  # u/v committed (in-order pipe)
                nc.vector.tensor_add(ss, sb_U[:], sb_Vt[:]).then_inc(sem_cp)
                # true cell state for next step; hides under ACT tanh_c
                vector.wait_ge(sem_cp, t + 1)
                nc.vector.tensor_scalar_mul(
                    sb_C[:, (t + 1) * 32:(t + 2) * 32], ss, 0.5
                ).then_inc(sem_cv)
                # h2 = (to+1)(.)tanh(c') = 2h; h-consumers use halved weights
                vector.wait_ge(sem_act, 3 * t + 3)
                nc.vector.scalar_tensor_tensor(
                    sb_H[:, (t + 1) * 32:(t + 2) * 32], gs[:, 96:128], 1.0,
                    sb_TC[:, s * 32:(s + 1) * 32], Alu.add, Alu.mult
                ).then_inc(sem_h)
                if t < M_TILES:
                    pre_copy(M_TILES + t)
                if t == TH + 2:
                    vector.wait_ge(dma_bo, 16)
                    vector.wait_ge(sem_hd, K2)
                    nc.vector.tensor_scalar_add(
                        sb_head[:, 0:HALF], ps_hd[0][0:11, 0:HALF],
                        sb_bob[0:11, 0:1]).then_inc(sem_hdcp)
            # ---- head copy, second half ----
            vector.wait_ge(sem_hd, 2 * K2)
            nc.vector.tensor_scalar_add(
                sb_head[:, HALF:2 * HALF], ps_hd[1][0:11, 0:HALF],
                sb_bob[0:11, 0:1]).then_inc(sem_hdcp)

        @block.scalar
        def _(scalar):
            Tanh = mybir.ActivationFunctionType.Tanh
            for t in range(T):
                s = t % 2
                gs = sb_G[:, s * 128:(s + 1) * 128]
                scalar.wait_ge(sem_pe, 2 * t + 1)
                scalar.activation(gs[:, 0:96], ps_z[s][:, 0:96], Tanh
                                  ).then_inc(sem_act)
                scalar.wait_ge(sem_pe, 2 * t + 2)
                scalar.activation(gs[:, 96:128], ps_z[s][:, 96:128], Tanh
                                  ).then_inc(sem_act)
                scalar.wait_ge(sem_cp, t + 1)
                scalar.activation(sb_TC[:, s * 32:(s + 1) * 32],
                                  ps_s[:, s * 32:(s + 1) * 32], Tanh,
                                  scale=0.5).then_inc(sem_act)

    return nc, ctx


_BUILD_CACHE = {}


def _get_nc(T):
    if T not in _BUILD_CACHE:
        _BUILD_CACHE[T] = _build(T)
    return _BUILD_CACHE[T][0]


def _prep_inputs(X, u, Wk, Wr, b_lstm, Wo, bo, Wc, bc, T):
    """Build the 8 per-core input maps (numpy, host-side sharding)."""
    # column scaling: i,f,o gates get 0.5 (sigma(x) = (tanh(x/2)+1)/2);
    # row scaling: recurrent/head weights get 0.5 because h is stored as 2h.
    col_scale = np.ones((1, 1024), np.float32)
    col_scale[:, :512] = 0.5          # i, f
    col_scale[:, 768:] = 0.5          # o   (g stays unscaled)
    Wk_p = np.ascontiguousarray(Wk[:, GATE_PERM] * col_scale
                                ).astype(np.float16)
    Wr_p = (Wr[:, GATE_PERM].astype(np.float32) * col_scale) * 0.5
    WrT = np.ascontiguousarray(
        Wr_p.reshape(2, 128, 1024).transpose(1, 0, 2).reshape(128, 2048)
    ).astype(np.float16)
    blstm = np.ascontiguousarray(
        (b_lstm[GATE_PERM].astype(np.float32) * col_scale[0]
         ).reshape(8, 128).T)
    WoC = np.concatenate([Wo.astype(np.float32),
                          Wc[:256].astype(np.float32)], axis=1) * 0.5
    WoC = np.ascontiguousarray(
        WoC.reshape(2, 128, 11).transpose(1, 0, 2).reshape(128, 22)
    ).astype(np.float16)
    bob = np.concatenate([bo.astype(np.float32), [0.0]]).reshape(11, 1)
    bob = np.ascontiguousarray(bob, np.float32)

    ident = np.eye(128, dtype=np.float16)
    in_maps = []
    for i in range(NCORES):
        bsl = slice(i * BL, (i + 1) * BL)
        Xt = np.ascontiguousarray(
            X[bsl, :T, :].astype(np.float32).transpose(2, 1, 0)
            .reshape(128, T * BL)).astype(np.float16)
        in_maps.append({
            "Xt": Xt, "WkT": Wk_p, "WrT": WrT, "blstm": blstm,
            "WoC": WoC, "bob": bob, "ident": ident,
        })
    return in_maps


def _sigmoid64(x):
    return 1.0 / (1.0 + np.exp(-x.astype(np.float64)))


def _softmax32(x):
    x = x.astype(np.float32)
    e = np.exp(x - x.max(axis=-1, keepdims=True))
    return (e / e.sum(axis=-1, keepdims=True)).astype(np.float32)


def _fallback_scan(x_seq, u_seq, h0, c0, t0, Wk, Wr, b_lstm, Wo, bo, Wc, bc):
    """Continue the reference recurrence on host for one sample that did not
    halt by t0.  Returns the sample's output row (float32)."""
    h = h0.astype(np.float32).copy()
    c = c0.astype(np.float32).copy()
    Wk = Wk.astype(np.float32); Wr = Wr.astype(np.float32)
    b_lstm = b_lstm.astype(np.float32)
    sig = lambda v: 1.0 / (1.0 + np.exp(-v))
    Tt = x_seq.shape[0]
    logits_last = None
    for t in range(t0, Tt):
        z = x_seq[t] @ Wk + h @ Wr + b_lstm
        i, f, g, o = np.split(z, 4)
        i = sig(i); f = sig(f); g = np.tanh(g); o = sig(o)
        c = f * c + i * g
        h = o * np.tanh(c)
        y = h @ Wo.astype(np.float32) + bo.astype(np.float32)
        logits = _softmax32(y)
        pre = float(h @ Wc[:256, 0].astype(np.float32)) \
            + t * float(Wc[256, 0]) + float(bc[0])
        probs = (1.0 - EPS) * sig(np.float32(pre)) + EPS * 0.05
        if u_seq[t] < probs:
            return logits
        logits_last = logits
    return logits_last


def kernel(**inputs):
    X = np.asarray(inputs["X"], np.float32)
    u = np.asarray(inputs["u"], np.float32)
    Wk = np.asarray(inputs["Wk"], np.float32)
    Wr = np.asarray(inputs["Wr"], np.float32)
    b_lstm = np.asarray(inputs["b_lstm"], np.float32)
    Wo = np.asarray(inputs["Wo"], np.float32)
    bo = np.asarray(inputs["bo"], np.float32)
    Wc = np.asarray(inputs["Wc"], np.float32)
    bc = np.asarray(inputs["bc"], np.float32)
    T = T_EFF

    nc = _get_nc(T)
    in_maps = _prep_inputs(X, u, Wk, Wr, b_lstm, Wo, bo, Wc, bc, T)
    res = run_bass_kernel_spmd(nc, in_maps, list(range(NCORES)))

    wc_t = float(Wc[256, 0])
    bias_c = float(bc[0])
    tvec = np.arange(T, dtype=np.float64)

    out = np.zeros((B, C), np.float32)
    for i in range(NCORES):
        bsl = slice(i * BL, (i + 1) * BL)
        head = res.results[i]["head"]          # [11, T*BL]
        y_pre = head[0:10].reshape(10, T, BL).transpose(1, 2, 0)  # [T, b, 10]
        pre_c = head[10].reshape(T, BL).astype(np.float64)        # [T, b]
        probs = (1.0 - EPS) * _sigmoid64(pre_c + tvec[:, None] * wc_t + bias_c) \
            + EPS * 0.05
        u_core = u[bsl, :T, 0]                 # [b, T]
        a = u_core.T.astype(np.float64) < probs  # [T, b]
        halted = a.any(axis=0)
        tstar = np.argmax(a, axis=0)           # first halt step per sample
        logits = _softmax32(y_pre)             # [T, b, 10]
        for b_ in range(BL):
            if halted[b_]:
                out[i * BL + b_] = logits[tstar[b_], b_]
            else:
                sh = res.results[i]["state_h"].astype(np.float32) * 0.5
                sc = res.results[i]["state_c"].astype(np.float32)
                h_T = sh.reshape(128, 2, BL).transpose(2, 1, 0) \
                    .reshape(BL, 256)[b_]
                c_T = sc.reshape(128, 2, BL).transpose(2, 1, 0) \
                    .reshape(BL, 256)[b_]
                out[i * BL + b_] = _fallback_scan(
                    X[i * BL + b_], u[i * BL + b_, :, 0], h_T, c_T, T,
                    Wk, Wr, b_lstm, Wo, bo, Wc, bc)
    return out



# revision 19
# speedup vs baseline: 2.9001x; 2.9001x over previous
# Trainium2 Bass kernel for nn_EARLIEST (adaptive-halting LSTM, B=128 T=4096
# V=128 H=256 C=10).
#
# The model halts each batch sample at the first step t where u[b,t] <
# probs[b,t] with probs ~= 0.45, so nearly every sample halts within a dozen
# steps.  The device runs the LSTM scan for T_EFF timesteps and streams the
# hidden-state history h(1..T_EFF) plus the final cell state back to the
# host.  The host computes the (tiny) output/halting heads from the history,
# applies the exact halting latch, and finishes any sample that has not
# halted by T_EFF with a numpy continuation of the recurrence — which keeps
# the kernel correct for arbitrary inputs while the device only pays for the
# steps that matter.
#
# Sharding: data-parallel over batch, 16 samples per core, weights
# replicated.  Layout is feature-major: h^T is [H=256, b=16] stored as two
# 128-partition k-tiles side by side so the recurrent matmuls need no
# transposes.  Gate order on device is (g, i, f, o) so the tanh gate can be
# activated early while the sigmoid gates stream.
#
# Per-step critical path:
#   DVE h -> PE 12x(LDW+MM) -> ACT sig(i,f) -> DVE u,v,s -> ACT tanh(c)
#   -> DVE h.  Semaphore waits are attached directly to the consuming
#   instructions where the instruction's input data is already resident
#   (lets the PE prefetch LDWEIGHTS past a waiting matmul); waits that gate
#   SBUF weight data (input DMAs) are standalone so they block LDWEIGHTS.

import numpy as np

import concourse.bass as bass
import concourse.mybir as mybir
from concourse.bass_utils import run_bass_kernel_spmd

B, T_FULL, V, H, C = 128, 4096, 128, 256, 10
EPS = 0.1
NCORES = 8
BL = B // NCORES  # 16 samples per core
T_EFF = 12
M_TILES = 8   # 4H/128
K2 = 2        # H/128
F32 = mybir.dt.float32
F16 = mybir.dt.float16

# device gate order (g, i, f, o); reference order is (i, f, g, o)
GATE_PERM = np.concatenate([
    np.arange(512, 768),    # g
    np.arange(0, 256),      # i
    np.arange(256, 512),    # f
    np.arange(768, 1024),   # o
])

CHUNK = 2  # timesteps per XW precompute chunk
USE_POOL_DMA = False
ATTACH_WAITS = False


def _gated(eng, sem, val, make):
    """Gate an instruction on sem>=val: attached to the instruction itself
    (pre-decoded in the wait queue) or as a standalone engine wait."""
    if ATTACH_WAITS:
        return make().wait_op(sem, val, "sem-ge")
    eng.wait_ge(sem, val)
    return make()


def _build(T, has_bias):
    """Raw-bass single-core program (SPMD across 8 cores)."""
    assert T % CHUNK == 0
    NCH = T // CHUNK
    nc = bass.Bass()

    d_Xt = nc.dram_tensor("Xt", [128, T * BL], F16, kind="ExternalInput")
    d_WkT = nc.dram_tensor("WkT", [128, 1024], F16, kind="ExternalInput")
    d_WrT = nc.dram_tensor("WrT", [128, 2048], F16, kind="ExternalInput")
    d_ident = nc.dram_tensor("ident", [128, 128], F16, kind="ExternalInput")
    if has_bias:
        d_blstm = nc.dram_tensor("blstm", [128, 8], F32, kind="ExternalInput")
    d_H = nc.dram_tensor("Hout", [128, T * 32], F16, kind="ExternalOutput")
    d_c = nc.dram_tensor("cout", [128, 32], F32, kind="ExternalOutput")

    from contextlib import ExitStack
    ctx = ExitStack()
    sb_Xt = ctx.enter_context(nc.sbuf_tensor([128, T * BL], F16))
    sb_WkT = ctx.enter_context(nc.sbuf_tensor([128, 1024], F16))
    sb_WrT = ctx.enter_context(nc.sbuf_tensor([128, 2048], F16))
    sb_I = ctx.enter_context(nc.sbuf_tensor([128, 128], F16))
    if has_bias:
        sb_blstm = ctx.enter_context(nc.sbuf_tensor([128, 8], F32))
    sb_XW = ctx.enter_context(nc.sbuf_tensor([128, T * 128], F16))
    sb_H = ctx.enter_context(nc.sbuf_tensor([128, (T + 1) * 32], F16))
    sb_G = ctx.enter_context(nc.sbuf_tensor([128, 2 * 128], F32))
    sb_TC = ctx.enter_context(nc.sbuf_tensor([128, 2 * 32], F32))
    sb_U = ctx.enter_context(nc.sbuf_tensor([128, 32], F32))
    sb_V = ctx.enter_context(nc.sbuf_tensor([128, 32], F32))
    sb_c = ctx.enter_context(nc.sbuf_tensor([128, 32], F32))

    # PSUM bank discipline: any engine READ of a bank must be semaphore-
    # ordered after the last PE WRITE into that bank (concurrent PE-W +
    # engine-R on one bank aborts with a PSUM collision).  Each gate group
    # gets its own ping-pong banks so its activation can fire the moment its
    # own matmuls retire while PE streams into the other banks.
    ps_zg = [ctx.enter_context(nc.psum_tensor(f"ps_zg{j}", [128, 512], F32))
             for j in range(2)]
    ps_zif = [ctx.enter_context(nc.psum_tensor(f"ps_zif{j}", [128, 512], F32))
              for j in range(2)]
    ps_zo = [ctx.enter_context(nc.psum_tensor(f"ps_zo{j}", [128, 512], F32))
             for j in range(2)]
    ps_s = ctx.enter_context(nc.psum_tensor("ps_s", [128, 512], F32))
    ps_pre = ctx.enter_context(nc.psum_tensor("ps_pre", [128, 512], F32))

    dma_xt = ctx.enter_context(nc.semaphore("dma_xt"))
    dma_wk = ctx.enter_context(nc.semaphore("dma_wk"))
    dma_id = ctx.enter_context(nc.semaphore("dma_id"))
    if has_bias:
        dma_bl = ctx.enter_context(nc.semaphore("dma_bl"))
    dma_p = ctx.enter_context(nc.semaphore("dma_p"))    # pool-queue inputs
    dma_out = ctx.enter_context(nc.semaphore("dma_out"))
    sem_premm = ctx.enter_context(nc.semaphore("sem_premm"))  # XW chunk mms
    sem_xw = ctx.enter_context(nc.semaphore("sem_xw"))        # XW chunk copies
    sem_h = ctx.enter_context(nc.semaphore("sem_h"))
    sem_pe = ctx.enter_context(nc.semaphore("sem_pe"))
    sem_act = ctx.enter_context(nc.semaphore("sem_act"))
    sem_uv = ctx.enter_context(nc.semaphore("sem_uv"))
    sem_s = ctx.enter_context(nc.semaphore("sem_s"))
    sem_cv = ctx.enter_context(nc.semaphore("sem_cv"))

    CB = CHUNK * BL  # psum cols per m-tile in an XW chunk

    with nc.Block() as block:

        @block.sync
        def _(sync):
            sync.dma_start(out=sb_Xt[:], in_=d_Xt[:]).then_inc(dma_xt, 16)
            if not USE_POOL_DMA:
                sync.dma_start(out=sb_WrT[:], in_=d_WrT[:]
                               ).then_inc(dma_p, 16)
            sync.dma_start(out=sb_WkT[:], in_=d_WkT[:]).then_inc(dma_wk, 16)
            sync.dma_start(out=sb_I[:], in_=d_ident[:]).then_inc(dma_id, 16)
            if has_bias:
                sync.dma_start(out=sb_blstm[:], in_=d_blstm[:]
                               ).then_inc(dma_bl, 16)
            # output: whole h history in one DMA once the scan retires
            sync.wait_ge(sem_h, T + 1)
            sync.dma_start(out=d_H[:], in_=sb_H[:, 32:(T + 1) * 32]
                           ).then_inc(dma_out, 16)
            sync.wait_ge(sem_cv, 1)
            sync.dma_start(out=d_c[:], in_=sb_c[:]).then_inc(dma_out, 16)
            sync.wait_ge(dma_out, 32)

        if USE_POOL_DMA:
            @block.gpsimd
            def _(gpsimd):
                gpsimd.dma_start(out=sb_WrT[:], in_=d_WrT[:]
                                 ).then_inc(dma_p, 16)

        @block.tensor
        def _(tensor):
            def pre_mm(c):
                # XW chunk c: 8 matmuls Wk_m^T @ Xt[chunk].  Single psum
                # bank: chunk c's mms wait for chunk c-1's copy to retire.
                for m in range(M_TILES):
                    make = lambda: tensor.matmul(
                        ps_pre[:, m * CB:(m + 1) * CB],
                        sb_WkT[:, m * 128:(m + 1) * 128],
                        sb_Xt[:, c * CB:(c + 1) * CB],
                        start=True, stop=True, skip_group_check=True,
                    )
                    if m == 0 and c >= 1:
                        mm = _gated(tensor, sem_xw, c, make)  # bank reuse
                    else:
                        mm = make()
                mm.then_inc(sem_premm)

            tensor.wait_ge(dma_xt, 16)    # gates LDW stream too
            tensor.wait_ge(dma_wk, 16)
            pre_mm(0)
            pre_mm(1)
            tensor.wait_ge(dma_id, 16)
            tensor.wait_ge(dma_p, 16)     # WrT
            tensor.wait_ge(sem_xw, 1)     # XW chunk 0 in SBUF

            for t in range(T):
                s2 = t % 2
                xwt = sb_XW[:, t * 128:(t + 1) * 128]
                # prefill z banks with XW[t]; off the critical path (no h
                # dep).  Bank reuse: A1-A3 of step t-2 consumed them (A3
                # last on the in-order ACT queue covers all three).
                if t >= 2:
                    if t % CHUNK == 0:
                        tensor.wait_ge(sem_xw, t // CHUNK + 1)
                    _gated(tensor, sem_act, 4 * (t - 2) + 3, lambda:
                           tensor.matmul(ps_zg[s2][:, 0:32], sb_I[:],
                                         xwt[:, 0:32], start=True, stop=True,
                                         skip_group_check=True))
                else:
                    tensor.matmul(ps_zg[s2][:, 0:32], sb_I[:], xwt[:, 0:32],
                                  start=True, stop=True,
                                  skip_group_check=True)
                tensor.matmul(ps_zif[s2][:, 0:64], sb_I[:], xwt[:, 32:96],
                              start=True, stop=True, skip_group_check=True)
                tensor.matmul(ps_zo[s2][:, 0:32], sb_I[:], xwt[:, 96:128],
                              start=True, stop=True, skip_group_check=True)
                # recurrent matmuls, gate order g,i,f,o; k inner
                first = True
                for m in range(M_TILES):
                    if m < 2:
                        out = ps_zg[s2][:, m * BL:(m + 1) * BL]
                    elif m < 6:
                        out = ps_zif[s2][:, (m - 2) * BL:(m - 1) * BL]
                    else:
                        out = ps_zo[s2][:, (m - 6) * BL:(m - 5) * BL]
                    for k in range(K2):
                        make = lambda: tensor.matmul(
                            out,
                            sb_WrT[:, k * 1024 + m * 128:
                                   k * 1024 + (m + 1) * 128],
                            sb_H[:, t * 32 + k * BL:t * 32 + (k + 1) * BL],
                            start=False, stop=False, skip_group_check=True,
                        )
                        if first:
                            mm = _gated(tensor, sem_h, t + 1, make)
                            first = False
                        else:
                            mm = make()
                    if m == 1 or m == 5 or m == 7:
                        mm.then_inc(sem_pe)   # g / i,f / o complete
                if t < NCH - 2:
                    pre_mm(t + 2)

        @block.scalar
        def _(scalar):
            Tanh = mybir.ActivationFunctionType.Tanh
            Sig = mybir.ActivationFunctionType.Sigmoid
            for t in range(T):
                s2 = t % 2
                gs = sb_G[:, s2 * 128:(s2 + 1) * 128]
                # A1: tanh(g) — fires after 4 matmuls, hides under PE stream
                _gated(scalar, sem_pe, 3 * t + 1, lambda: scalar.activation(
                    gs[:, 0:32], ps_zg[s2][:, 0:32], Tanh)).then_inc(sem_act)
                # A2: sigmoid(i,f)
                _gated(scalar, sem_pe, 3 * t + 2, lambda: scalar.activation(
                    gs[:, 32:96], ps_zif[s2][:, 0:64], Sig)).then_inc(sem_act)
                # A3: sigmoid(o)
                _gated(scalar, sem_pe, 3 * t + 3, lambda: scalar.activation(
                    gs[:, 96:128], ps_zo[s2][:, 0:32], Sig)).then_inc(sem_act)
                # A4: tanh(c')
                _gated(scalar, sem_s, t + 1, lambda: scalar.activation(
                    sb_TC[:, s2 * 32:(s2 + 1) * 32],
                    ps_s[:, s2 * 32:(s2 + 1) * 32], Tanh)).then_inc(sem_act)

        @block.vector
        def _(vector):
            xw4 = sb_XW[:].rearrange("p (t m b) -> p t m b", m=M_TILES, b=BL)

            def pre_copy(c):
                # psum chunk (m-major) -> sb_XW (t-major) fp16 (+ bias)
                first = True
                for m in range(M_TILES):
                    psrc = ps_pre[:, m * CB:(m + 1) * CB] \
                        .rearrange("p (t b) -> p t b", b=BL)
                    dst = xw4[:, c * CHUNK:(c + 1) * CHUNK, m, :]
                    if has_bias:
                        make = lambda: nc.vector.tensor_scalar_add(
                            dst, psrc, sb_blstm[:, m:m + 1])
                    else:
                        make = lambda: nc.vector.tensor_copy(dst, psrc)
                    if first:
                        op = _gated(vector, sem_premm, c + 1, make)
                        first = False
                    else:
                        op = make()
                op.then_inc(sem_xw)

            vector.memset(sb_H[:, 0:32], 0.0).then_inc(sem_h)
            if has_bias:
                vector.wait_ge(dma_bl, 16)
            pre_copy(0)
            pre_copy(1)
            vector.drain()

            for t in range(T):
                s2 = t % 2
                gs = sb_G[:, s2 * 128:(s2 + 1) * 128]
                ss = ps_s[:, s2 * 32:(s2 + 1) * 32]
                cprev = ps_s[:, (1 - s2) * 32:(2 - s2) * 32]
                if t == 0:
                    # c0 = 0: c1 = i*g directly into psum
                    _gated(vector, sem_act, 4 * t + 2, lambda:
                           nc.vector.tensor_mul(ss, gs[:, 32:64], gs[:, 0:32])
                           ).then_inc(sem_s)
                else:
                    # u = i*g
                    _gated(vector, sem_act, 4 * t + 2, lambda:
                           nc.vector.tensor_mul(sb_U[:], gs[:, 32:64],
                                                gs[:, 0:32]))
                    # v = f*c   (bank-reuse guard: A4(t-2) has read ps_s)
                    if t >= 2:
                        vop = _gated(vector, sem_act, 4 * (t - 2) + 4,
                                     lambda: nc.vector.tensor_mul(
                                         sb_V[:], gs[:, 64:96], cprev))
                    else:
                        vop = nc.vector.tensor_mul(sb_V[:], gs[:, 64:96],
                                                   cprev)
                    vop.then_inc(sem_uv)
                    # c' = u + v
                    _gated(vector, sem_uv, t, lambda:
                           nc.vector.tensor_add(ss, sb_U[:], sb_V[:])
                           ).then_inc(sem_s)
                # h = o * tanh(c')
                _gated(vector, sem_act, 4 * t + 4, lambda:
                       nc.vector.tensor_mul(
                           sb_H[:, (t + 1) * 32:(t + 2) * 32], gs[:, 96:128],
                           sb_TC[:, s2 * 32:(s2 + 1) * 32])).then_inc(sem_h)
                if t < NCH - 2:
                    pre_copy(t + 2)
            # final cell state for the host fallback
            nc.vector.tensor_scalar_mul(
                sb_c[:], ps_s[:, ((T - 1) % 2) * 32:((T - 1) % 2 + 1) * 32],
                1.0).then_inc(sem_cv)

    return nc, ctx


_BUILD_CACHE = {}


def _get_nc(T, has_bias):
    key = (T, has_bias)
    if key not in _BUILD_CACHE:
        _BUILD_CACHE[key] = _build(T, has_bias)
    return _BUILD_CACHE[key][0]


def _prep_inputs(X, Wk, Wr, b_lstm, T, has_bias):
    """Build the 8 per-core input maps (numpy, host-side sharding)."""
    Wk_p = np.ascontiguousarray(Wk[:, GATE_PERM]).astype(np.float16)
    Wr_p = Wr[:, GATE_PERM].astype(np.float32)
    WrT = np.ascontiguousarray(
        Wr_p.reshape(2, 128, 1024).transpose(1, 0, 2).reshape(128, 2048)
    ).astype(np.float16)
    ident = np.eye(128, dtype=np.float16)
    base = {"WkT": Wk_p, "WrT": WrT, "ident": ident}
    if has_bias:
        base["blstm"] = np.ascontiguousarray(
            b_lstm[GATE_PERM].astype(np.float32).reshape(8, 128).T)
    in_maps = []
    for i in range(NCORES):
        bsl = slice(i * BL, (i + 1) * BL)
        Xt = np.ascontiguousarray(
            X[bsl, :T, :].astype(np.float32).transpose(2, 1, 0)
            .reshape(128, T * BL)).astype(np.float16)
        m = dict(base)
        m["Xt"] = Xt
        in_maps.append(m)
    return in_maps


def _sigmoid64(x):
    return 1.0 / (1.0 + np.exp(-x.astype(np.float64)))


def _softmax32(x):
    x = x.astype(np.float32)
    e = np.exp(x - x.max(axis=-1, keepdims=True))
    return (e / e.sum(axis=-1, keepdims=True)).astype(np.float32)


def _fallback_scan(x_seq, u_seq, h0, c0, t0, Wk, Wr, b_lstm, Wo, bo, Wc, bc):
    """Continue the reference recurrence on host for one sample that did not
    halt by t0.  Returns the sample's output row (float32)."""
    h = h0.astype(np.float32).copy()
    c = c0.astype(np.float32).copy()
    Wk = Wk.astype(np.float32); Wr = Wr.astype(np.float32)
    b_lstm = b_lstm.astype(np.float32)
    sig = lambda v: 1.0 / (1.0 + np.exp(-v))
    Tt = x_seq.shape[0]
    logits_last = None
    for t in range(t0, Tt):
        z = x_seq[t] @ Wk + h @ Wr + b_lstm
        i, f, g, o = np.split(z, 4)
        i = sig(i); f = sig(f); g = np.tanh(g); o = sig(o)
        c = f * c + i * g
        h = o * np.tanh(c)
        y = h @ Wo.astype(np.float32) + bo.astype(np.float32)
        logits = _softmax32(y)
        pre = float(h @ Wc[:256, 0].astype(np.float32)) \
            + t * float(Wc[256, 0]) + float(bc[0])
        probs = (1.0 - EPS) * sig(np.float32(pre)) + EPS * 0.05
        if u_seq[t] < probs:
            return logits
        logits_last = logits
    return logits_last


def kernel(**inputs):
    X = np.asarray(inputs["X"], np.float32)
    u = np.asarray(inputs["u"], np.float32)
    Wk = np.asarray(inputs["Wk"], np.float32)
    Wr = np.asarray(inputs["Wr"], np.float32)
    b_lstm = np.asarray(inputs["b_lstm"], np.float32)
    Wo = np.asarray(inputs["Wo"], np.float32)
    bo = np.asarray(inputs["bo"], np.float32)
    Wc = np.asarray(inputs["Wc"], np.float32)
    bc = np.asarray(inputs["bc"], np.float32)
    T = T_EFF
    has_bias = bool(np.any(b_lstm))

    nc = _get_nc(T, has_bias)
    in_maps = _prep_inputs(X, Wk, Wr, b_lstm, T, has_bias)
    res = run_bass_kernel_spmd(nc, in_maps, list(range(NCORES)))

    wc_t = float(Wc[256, 0])
    bias_c = float(bc[0])
    tvec = np.arange(T, dtype=np.float64)
    Wo64 = Wo.astype(np.float64)
    Wc64 = Wc[:256, 0].astype(np.float64)

    out = np.zeros((B, C), np.float32)
    for i in range(NCORES):
        bsl = slice(i * BL, (i + 1) * BL)
        hraw = res.results[i]["Hout"]         # [128, T*32] fp16
        # cols: t*32 + k*16 + b ; partitions: feature within k-tile
        h_hist = hraw.reshape(128, T, 2, BL).transpose(1, 3, 2, 0) \
            .reshape(T, BL, 256).astype(np.float64)   # h after step t
        y = h_hist @ Wo64 + bo.astype(np.float64)     # [T, b, C]
        pre_c = h_hist @ Wc64 + tvec[:, None] * wc_t + bias_c  # [T, b]
        probs = (1.0 - EPS) * _sigmoid64(pre_c) + EPS * 0.05
        u_core = u[bsl, :T, 0]                 # [b, T]
        a = u_core.T.astype(np.float64) < probs  # [T, b]
        halted = a.any(axis=0)
        tstar = np.argmax(a, axis=0)
        logits = _softmax32(y)                 # [T, b, C]
        craw = res.results[i]["cout"]          # [128, 32] fp32
        c_T = craw.reshape(128, 2, BL).transpose(2, 1, 0).reshape(BL, 256)
        for b_ in range(BL):
            if halted[b_]:
                out[i * BL + b_] = logits[tstar[b_], b_]
            else:
                out[i * BL + b_] = _fallback_scan(
                    X[i * BL + b_], u[i * BL + b_, :, 0],
                    h_hist[T - 1, b_].astype(np.float32), c_T[b_], T,
                    Wk, Wr, b_lstm, Wo, bo, Wc, bc)
    return out


# revision 21
# speedup vs baseline: 4.5412x; 1.5659x over previous
# Trainium2 Bass kernel for nn_EARLIEST (adaptive-halting LSTM, B=128 T=4096
# V=128 H=256 C=10).
#
# The model halts each batch sample at the first step t where u[b,t] <
# probs[b,t] with probs ~= 0.45, so nearly every sample halts within a dozen
# steps.  The device runs the LSTM scan for T_EFF timesteps and streams the
# hidden-state history h(1..T_EFF) plus the final cell state back to the
# host.  The host computes the (tiny) output/halting heads from the history,
# applies the exact halting latch, and finishes any sample that has not
# halted by T_EFF with a numpy continuation of the recurrence — which keeps
# the kernel correct for arbitrary inputs while the device only pays for the
# steps that matter.
#
# Sharding: data-parallel over batch, 16 samples per core, weights
# replicated.  Layout is feature-major: h^T is [H=256, b=16] stored as two
# 128-partition k-tiles side by side so the recurrent matmuls need no
# transposes.  Gate order on device is (g, i, f, o) so the tanh gate can be
# activated early while the sigmoid gates stream.
#
# Per-step critical path:
#   DVE h -> PE 12x(LDW+MM) -> ACT sig(i,f) -> DVE u,v,s -> ACT tanh(c)
#   -> DVE h.  Semaphore waits are attached directly to the consuming
#   instructions where the instruction's input data is already resident
#   (lets the PE prefetch LDWEIGHTS past a waiting matmul); waits that gate
#   SBUF weight data (input DMAs) are standalone so they block LDWEIGHTS.

import numpy as np

import concourse.bass as bass
import concourse.mybir as mybir
from concourse.bass_utils import run_bass_kernel_spmd

B, T_FULL, V, H, C = 128, 4096, 128, 256, 10
EPS = 0.1
NCORES = 8
BL = B // NCORES  # 16 samples per core
T_EFF = 8
M_TILES = 8   # 4H/128
K2 = 2        # H/128
F32 = mybir.dt.float32
F16 = mybir.dt.float16

# device gate order (g, i, f, o); reference order is (i, f, g, o)
GATE_PERM = np.concatenate([
    np.arange(512, 768),    # g
    np.arange(0, 256),      # i
    np.arange(256, 512),    # f
    np.arange(768, 1024),   # o
])

CHUNK = 2  # timesteps per XW precompute chunk
USE_POOL_DMA = True
ATTACH_WAITS = True


def _gated(eng, sem, val, make):
    """Gate an instruction on sem>=val: attached to the instruction itself
    (pre-decoded in the wait queue) or as a standalone engine wait."""
    if ATTACH_WAITS:
        return make().wait_op(sem, val, "sem-ge")
    eng.wait_ge(sem, val)
    return make()


def _build(T, has_bias):
    """Raw-bass single-core program (SPMD across 8 cores)."""
    assert T % CHUNK == 0
    NCH = T // CHUNK
    nc = bass.Bass()

    d_Xt = nc.dram_tensor("Xt", [128, T * BL], F16, kind="ExternalInput")
    d_WkT = nc.dram_tensor("WkT", [128, 1024], F16, kind="ExternalInput")
    d_WrT = nc.dram_tensor("WrT", [128, 2048], F16, kind="ExternalInput")
    d_ident = nc.dram_tensor("ident", [128, 128], F16, kind="ExternalInput")
    if has_bias:
        d_blstm = nc.dram_tensor("blstm", [128, 8], F32, kind="ExternalInput")
    d_H = nc.dram_tensor("Hout", [128, T * 32], F16, kind="ExternalOutput")
    d_c = nc.dram_tensor("cout", [128, 32], F32, kind="ExternalOutput")

    from contextlib import ExitStack
    ctx = ExitStack()
    sb_Xt = ctx.enter_context(nc.sbuf_tensor([128, T * BL], F16))
    sb_WkT = ctx.enter_context(nc.sbuf_tensor([128, 1024], F16))
    sb_WrT = ctx.enter_context(nc.sbuf_tensor([128, 2048], F16))
    sb_I = ctx.enter_context(nc.sbuf_tensor([128, 128], F16))
    if has_bias:
        sb_blstm = ctx.enter_context(nc.sbuf_tensor([128, 8], F32))
    sb_XW = ctx.enter_context(nc.sbuf_tensor([128, T * 128], F16))
    sb_H = ctx.enter_context(nc.sbuf_tensor([128, (T + 1) * 32], F16))
    sb_G = ctx.enter_context(nc.sbuf_tensor([128, 2 * 128], F32))
    sb_TC = ctx.enter_context(nc.sbuf_tensor([128, 2 * 32], F32))
    sb_U = ctx.enter_context(nc.sbuf_tensor([128, 32], F32))
    sb_V = ctx.enter_context(nc.sbuf_tensor([128, 32], F32))
    sb_c = ctx.enter_context(nc.sbuf_tensor([128, 32], F32))

    # PSUM bank discipline: any engine READ of a bank must be semaphore-
    # ordered after the last PE WRITE into that bank (concurrent PE-W +
    # engine-R on one bank aborts with a PSUM collision).  Each gate group
    # gets its own ping-pong banks so its activation can fire the moment its
    # own matmuls retire while PE streams into the other banks.
    ps_zg = [ctx.enter_context(nc.psum_tensor(f"ps_zg{j}", [128, 512], F32))
             for j in range(2)]
    ps_zif = [ctx.enter_context(nc.psum_tensor(f"ps_zif{j}", [128, 512], F32))
              for j in range(2)]
    ps_zo = [ctx.enter_context(nc.psum_tensor(f"ps_zo{j}", [128, 512], F32))
             for j in range(2)]
    ps_s = ctx.enter_context(nc.psum_tensor("ps_s", [128, 512], F32))
    ps_pre = ctx.enter_context(nc.psum_tensor("ps_pre", [128, 512], F32))

    dma_xt = ctx.enter_context(nc.semaphore("dma_xt"))
    dma_wk = ctx.enter_context(nc.semaphore("dma_wk"))
    dma_id = ctx.enter_context(nc.semaphore("dma_id"))
    if has_bias:
        dma_bl = ctx.enter_context(nc.semaphore("dma_bl"))
    dma_p = ctx.enter_context(nc.semaphore("dma_p"))    # pool-queue inputs
    dma_out = ctx.enter_context(nc.semaphore("dma_out"))
    sem_premm = ctx.enter_context(nc.semaphore("sem_premm"))  # XW chunk mms
    sem_xw = ctx.enter_context(nc.semaphore("sem_xw"))        # XW chunk copies
    sem_h = ctx.enter_context(nc.semaphore("sem_h"))
    sem_pe = ctx.enter_context(nc.semaphore("sem_pe"))
    sem_act = ctx.enter_context(nc.semaphore("sem_act"))
    sem_uv = ctx.enter_context(nc.semaphore("sem_uv"))
    sem_s = ctx.enter_context(nc.semaphore("sem_s"))
    sem_cv = ctx.enter_context(nc.semaphore("sem_cv"))

    CB = CHUNK * BL  # psum cols per m-tile in an XW chunk

    with nc.Block() as block:

        @block.sync
        def _(sync):
            sync.dma_start(out=sb_WkT[:], in_=d_WkT[:]).then_inc(dma_wk, 16)
            if not USE_POOL_DMA:
                sync.dma_start(out=sb_WrT[:], in_=d_WrT[:]
                               ).then_inc(dma_p, 16)
            if has_bias:
                sync.dma_start(out=sb_blstm[:], in_=d_blstm[:]
                               ).then_inc(dma_bl, 16)
            # output: whole h history in one DMA once the scan retires
            sync.wait_ge(sem_h, T + 1)
            sync.dma_start(out=d_H[:], in_=sb_H[:, 32:(T + 1) * 32]
                           ).then_inc(dma_out, 16)
            sync.wait_ge(sem_cv, 1)
            sync.dma_start(out=d_c[:], in_=sb_c[:]).then_inc(dma_out, 16)
            sync.wait_ge(dma_out, 32)

        if USE_POOL_DMA:
            @block.gpsimd
            def _(gpsimd):
                gpsimd.dma_start(out=sb_WrT[:], in_=d_WrT[:]
                                 ).then_inc(dma_p, 16)

        @block.tensor
        def _(tensor):
            def pre_mm(c):
                # XW chunk c: 8 matmuls Wk_m^T @ Xt[chunk].  Single psum
                # bank: chunk c's mms wait for chunk c-1's copy to retire.
                for m in range(M_TILES):
                    make = lambda: tensor.matmul(
                        ps_pre[:, m * CB:(m + 1) * CB],
                        sb_WkT[:, m * 128:(m + 1) * 128],
                        sb_Xt[:, c * CB:(c + 1) * CB],
                        start=True, stop=True, skip_group_check=True,
                    )
                    if m == 0 and c >= 1:
                        mm = _gated(tensor, sem_xw, c, make)  # bank reuse
                    else:
                        mm = make()
                mm.then_inc(sem_premm)

            tensor.wait_ge(dma_xt, 16)    # gates LDW stream too
            tensor.wait_ge(dma_wk, 16)
            pre_mm(0)
            pre_mm(1)
            tensor.wait_ge(dma_id, 16)
            tensor.wait_ge(dma_p, 16)     # WrT
            tensor.wait_ge(sem_xw, 1)     # XW chunk 0 in SBUF

            for t in range(T):
                s2 = t % 2
                xwt = sb_XW[:, t * 128:(t + 1) * 128]
                # prefill z banks with XW[t]; off the critical path (no h
                # dep).  Bank reuse: A1-A3 of step t-2 consumed them (A3
                # last on the in-order ACT queue covers all three).
                if t >= 2:
                    if t % CHUNK == 0:
                        tensor.wait_ge(sem_xw, t // CHUNK + 1)
                    _gated(tensor, sem_act, 4 * (t - 2) + 3, lambda:
                           tensor.matmul(ps_zg[s2][:, 0:32], sb_I[:],
                                         xwt[:, 0:32], start=True, stop=True,
                                         skip_group_check=True))
                else:
                    tensor.matmul(ps_zg[s2][:, 0:32], sb_I[:], xwt[:, 0:32],
                                  start=True, stop=True,
                                  skip_group_check=True)
                tensor.matmul(ps_zif[s2][:, 0:64], sb_I[:], xwt[:, 32:96],
                              start=True, stop=True, skip_group_check=True)
                tensor.matmul(ps_zo[s2][:, 0:32], sb_I[:], xwt[:, 96:128],
                              start=True, stop=True, skip_group_check=True)
                # recurrent matmuls, gate order g,i,f,o; k inner
                first = True
                for m in range(M_TILES):
                    if m < 2:
                        out = ps_zg[s2][:, m * BL:(m + 1) * BL]
                    elif m < 6:
                        out = ps_zif[s2][:, (m - 2) * BL:(m - 1) * BL]
                    else:
                        out = ps_zo[s2][:, (m - 6) * BL:(m - 5) * BL]
                    for k in range(K2):
                        make = lambda: tensor.matmul(
                            out,
                            sb_WrT[:, k * 1024 + m * 128:
                                   k * 1024 + (m + 1) * 128],
                            sb_H[:, t * 32 + k * BL:t * 32 + (k + 1) * BL],
                            start=False, stop=False, skip_group_check=True,
                        )
                        if first:
                            mm = _gated(tensor, sem_h, t + 1, make)
                            first = False
                        else:
                            mm = make()
                    if m == 1 or m == 5 or m == 7:
                        mm.then_inc(sem_pe)   # g / i,f / o complete
                if t < NCH - 2:
                    pre_mm(t + 2)

        @block.scalar
        def _(scalar):
            Tanh = mybir.ActivationFunctionType.Tanh
            Sig = mybir.ActivationFunctionType.Sigmoid
            scalar.dma_start(out=sb_Xt[:], in_=d_Xt[:]).then_inc(dma_xt, 16)
            scalar.dma_start(out=sb_I[:], in_=d_ident[:]).then_inc(dma_id, 16)
            for t in range(T):
                s2 = t % 2
                gs = sb_G[:, s2 * 128:(s2 + 1) * 128]
                # A1: tanh(g) — fires after 4 matmuls, hides under PE stream
                _gated(scalar, sem_pe, 3 * t + 1, lambda: scalar.activation(
                    gs[:, 0:32], ps_zg[s2][:, 0:32], Tanh)).then_inc(sem_act)
                # A2: sigmoid(i,f)
                _gated(scalar, sem_pe, 3 * t + 2, lambda: scalar.activation(
                    gs[:, 32:96], ps_zif[s2][:, 0:64], Sig)).then_inc(sem_act)
                # A3: sigmoid(o)
                _gated(scalar, sem_pe, 3 * t + 3, lambda: scalar.activation(
                    gs[:, 96:128], ps_zo[s2][:, 0:32], Sig)).then_inc(sem_act)
                # A4: tanh(c')
                _gated(scalar, sem_s, t + 1, lambda: scalar.activation(
                    sb_TC[:, s2 * 32:(s2 + 1) * 32],
                    ps_s[:, s2 * 32:(s2 + 1) * 32], Tanh)).then_inc(sem_act)

        @block.vector
        def _(vector):
            xw4 = sb_XW[:].rearrange("p (t m b) -> p t m b", m=M_TILES, b=BL)

            def pre_copy(c):
                # psum chunk (m-major) -> sb_XW (t-major) fp16 (+ bias)
                first = True
                for m in range(M_TILES):
                    psrc = ps_pre[:, m * CB:(m + 1) * CB] \
                        .rearrange("p (t b) -> p t b", b=BL)
                    dst = xw4[:, c * CHUNK:(c + 1) * CHUNK, m, :]
                    if has_bias:
                        make = lambda: nc.vector.tensor_scalar_add(
                            dst, psrc, sb_blstm[:, m:m + 1])
                    else:
                        make = lambda: nc.vector.tensor_copy(dst, psrc)
                    if first:
                        op = _gated(vector, sem_premm, c + 1, make)
                        first = False
                    else:
                        op = make()
                op.then_inc(sem_xw)

            vector.memset(sb_H[:, 0:32], 0.0).then_inc(sem_h)
            if has_bias:
                vector.wait_ge(dma_bl, 16)
            pre_copy(0)
            pre_copy(1)
            vector.drain()

            for t in range(T):
                s2 = t % 2
                gs = sb_G[:, s2 * 128:(s2 + 1) * 128]
                ss = ps_s[:, s2 * 32:(s2 + 1) * 32]
                cprev = ps_s[:, (1 - s2) * 32:(2 - s2) * 32]
                if t == 0:
                    # c0 = 0: c1 = i*g directly into psum
                    _gated(vector, sem_act, 4 * t + 2, lambda:
                           nc.vector.tensor_mul(ss, gs[:, 32:64], gs[:, 0:32])
                           ).then_inc(sem_s)
                else:
                    # u = i*g
                    _gated(vector, sem_act, 4 * t + 2, lambda:
                           nc.vector.tensor_mul(sb_U[:], gs[:, 32:64],
                                                gs[:, 0:32]))
                    # v = f*c   (bank-reuse guard: A4(t-2) has read ps_s)
                    if t >= 2:
                        vop = _gated(vector, sem_act, 4 * (t - 2) + 4,
                                     lambda: nc.vector.tensor_mul(
                                         sb_V[:], gs[:, 64:96], cprev))
                    else:
                        vop = nc.vector.tensor_mul(sb_V[:], gs[:, 64:96],
                                                   cprev)
                    vop.then_inc(sem_uv)
                    # c' = u + v
                    _gated(vector, sem_uv, t, lambda:
                           nc.vector.tensor_add(ss, sb_U[:], sb_V[:])
                           ).then_inc(sem_s)
                # h = o * tanh(c')
                _gated(vector, sem_act, 4 * t + 4, lambda:
                       nc.vector.tensor_mul(
                           sb_H[:, (t + 1) * 32:(t + 2) * 32], gs[:, 96:128],
                           sb_TC[:, s2 * 32:(s2 + 1) * 32])).then_inc(sem_h)
                if t < NCH - 2:
                    pre_copy(t + 2)
            # final cell state for the host fallback
            nc.vector.tensor_scalar_mul(
                sb_c[:], ps_s[:, ((T - 1) % 2) * 32:((T - 1) % 2 + 1) * 32],
                1.0).then_inc(sem_cv)

    return nc, ctx


_BUILD_CACHE = {}


def _get_nc(T, has_bias):
    key = (T, has_bias)
    if key not in _BUILD_CACHE:
        _BUILD_CACHE[key] = _build(T, has_bias)
    return _BUILD_CACHE[key][0]


def _prep_inputs(X, Wk, Wr, b_lstm, T, has_bias):
    """Build the 8 per-core input maps (numpy, host-side sharding)."""
    Wk_p = np.ascontiguousarray(Wk[:, GATE_PERM]).astype(np.float16)
    Wr_p = Wr[:, GATE_PERM].astype(np.float32)
    WrT = np.ascontiguousarray(
        Wr_p.reshape(2, 128, 1024).transpose(1, 0, 2).reshape(128, 2048)
    ).astype(np.float16)
    ident = np.eye(128, dtype=np.float16)
    base = {"WkT": Wk_p, "WrT": WrT, "ident": ident}
    if has_bias:
        base["blstm"] = np.ascontiguousarray(
            b_lstm[GATE_PERM].astype(np.float32).reshape(8, 128).T)
    in_maps = []
    for i in range(NCORES):
        bsl = slice(i * BL, (i + 1) * BL)
        Xt = np.ascontiguousarray(
            X[bsl, :T, :].astype(np.float32).transpose(2, 1, 0)
            .reshape(128, T * BL)).astype(np.float16)
        m = dict(base)
        m["Xt"] = Xt
        in_maps.append(m)
    return in_maps


def _sigmoid64(x):
    return 1.0 / (1.0 + np.exp(-x.astype(np.float64)))


def _softmax32(x):
    x = x.astype(np.float32)
    e = np.exp(x - x.max(axis=-1, keepdims=True))
    return (e / e.sum(axis=-1, keepdims=True)).astype(np.float32)


def _fallback_scan(x_seq, u_seq, h0, c0, t0, Wk, Wr, b_lstm, Wo, bo, Wc, bc):
    """Continue the reference recurrence on host for one sample that did not
    halt by t0.  Returns the sample's output row (float32)."""
    h = h0.astype(np.float32).copy()
    c = c0.astype(np.float32).copy()
    Wk = Wk.astype(np.float32); Wr = Wr.astype(np.float32)
    b_lstm = b_lstm.astype(np.float32)
    sig = lambda v: 1.0 / (1.0 + np.exp(-v))
    Tt = x_seq.shape[0]
    logits_last = None
    for t in range(t0, Tt):
        z = x_seq[t] @ Wk + h @ Wr + b_lstm
        i, f, g, o = np.split(z, 4)
        i = sig(i); f = sig(f); g = np.tanh(g); o = sig(o)
        c = f * c + i * g
        h = o * np.tanh(c)
        y = h @ Wo.astype(np.float32) + bo.astype(np.float32)
        logits = _softmax32(y)
        pre = float(h @ Wc[:256, 0].astype(np.float32)) \
            + t * float(Wc[256, 0]) + float(bc[0])
        probs = (1.0 - EPS) * sig(np.float32(pre)) + EPS * 0.05
        if u_seq[t] < probs:
            return logits
        logits_last = logits
    return logits_last


def kernel(**inputs):
    X = np.asarray(inputs["X"], np.float32)
    u = np.asarray(inputs["u"], np.float32)
    Wk = np.asarray(inputs["Wk"], np.float32)
    Wr = np.asarray(inputs["Wr"], np.float32)
    b_lstm = np.asarray(inputs["b_lstm"], np.float32)
    Wo = np.asarray(inputs["Wo"], np.float32)
    bo = np.asarray(inputs["bo"], np.float32)
    Wc = np.asarray(inputs["Wc"], np.float32)
    bc = np.asarray(inputs["bc"], np.float32)
    T = T_EFF
    has_bias = bool(np.any(b_lstm))

    nc = _get_nc(T, has_bias)
    in_maps = _prep_inputs(X, Wk, Wr, b_lstm, T, has_bias)
    res = run_bass_kernel_spmd(nc, in_maps, list(range(NCORES)))

    wc_t = float(Wc[256, 0])
    bias_c = float(bc[0])
    tvec = np.arange(T, dtype=np.float64)
    Wo64 = Wo.astype(np.float64)
    Wc64 = Wc[:256, 0].astype(np.float64)

    out = np.zeros((B, C), np.float32)
    for i in range(NCORES):
        bsl = slice(i * BL, (i + 1) * BL)
        hraw = res.results[i]["Hout"]         # [128, T*32] fp16
        # cols: t*32 + k*16 + b ; partitions: feature within k-tile
        h_hist = hraw.reshape(128, T, 2, BL).transpose(1, 3, 2, 0) \
            .reshape(T, BL, 256).astype(np.float64)   # h after step t
        y = h_hist @ Wo64 + bo.astype(np.float64)     # [T, b, C]
        pre_c = h_hist @ Wc64 + tvec[:, None] * wc_t + bias_c  # [T, b]
        probs = (1.0 - EPS) * _sigmoid64(pre_c) + EPS * 0.05
        u_core = u[bsl, :T, 0]                 # [b, T]
        a = u_core.T.astype(np.float64) < probs  # [T, b]
        halted = a.any(axis=0)
        tstar = np.argmax(a, axis=0)
        logits = _softmax32(y)                 # [T, b, C]
        craw = res.results[i]["cout"]          # [128, 32] fp32
        c_T = craw.reshape(128, 2, BL).transpose(2, 1, 0).reshape(BL, 256)
        for b_ in range(BL):
            if halted[b_]:
                out[i * BL + b_] = logits[tstar[b_], b_]
            else:
                out[i * BL + b_] = _fallback_scan(
                    X[i * BL + b_], u[i * BL + b_, :, 0],
                    h_hist[T - 1, b_].astype(np.float32), c_T[b_], T,
                    Wk, Wr, b_lstm, Wo, bo, Wc, bc)
    return out


# revision 22
# speedup vs baseline: 5.3772x; 1.1841x over previous
# Trainium2 Bass kernel for nn_EARLIEST (adaptive-halting LSTM, B=128 T=4096
# V=128 H=256 C=10).
#
# The model halts each batch sample at the first step t where u[b,t] <
# probs[b,t] with probs ~= 0.45, so nearly every sample halts within a dozen
# steps.  The device runs the LSTM scan for T_EFF timesteps and streams the
# hidden-state history h(1..T_EFF) plus the final cell state back to the
# host.  The host computes the (tiny) output/halting heads from the history,
# applies the exact halting latch, and finishes any sample that has not
# halted by T_EFF with a numpy continuation of the recurrence — which keeps
# the kernel correct for arbitrary inputs while the device only pays for the
# steps that matter.
#
# Sharding: data-parallel over batch, 16 samples per core, weights
# replicated.  Layout is feature-major: h^T is [H=256, b=16] stored as two
# 128-partition k-tiles side by side so the recurrent matmuls need no
# transposes.  Gate order on device is (g, i, f, o).
#
# Per step each gate tile accumulates Wk_m^T x_t (issued before h is ready)
# plus the two Wr_mk^T h tiles directly in PSUM — there is no separate x-
# projection precompute.  PSUM bank discipline: an engine READ of a bank
# must be semaphore-ordered after the last PE WRITE to that bank (concurrent
# PE-W + engine-R on one bank is a fatal PSUM collision), so each gate group
# owns ping-pong bank pairs and its activation fires exactly when its own
# matmuls retire while PE streams into other banks.
#
# Per-step critical path:
#   DVE h -> PE 12x(LDW+MM) -> ACT sig(i,f) -> DVE u,v,s -> ACT tanh(c)
#   -> DVE h, with semaphore waits attached to the consuming instructions.

import numpy as np

import concourse.bass as bass
import concourse.mybir as mybir
from concourse.bass_utils import run_bass_kernel_spmd

B, T_FULL, V, H, C = 128, 4096, 128, 256, 10
EPS = 0.1
NCORES = 8
BL = B // NCORES  # 16 samples per core
T_EFF = 6
M_TILES = 8   # 4H/128
K2 = 2        # H/128
F32 = mybir.dt.float32
F16 = mybir.dt.float16

# device gate order (g, i, f, o); reference order is (i, f, g, o)
GATE_PERM = np.concatenate([
    np.arange(512, 768),    # g
    np.arange(0, 256),      # i
    np.arange(256, 512),    # f
    np.arange(768, 1024),   # o
])


def _build(T, has_bias):
    """Raw-bass single-core program (SPMD across 8 cores)."""
    nc = bass.Bass()

    d_Xt = nc.dram_tensor("Xt", [128, T * BL], F16, kind="ExternalInput")
    d_WkT = nc.dram_tensor("WkT", [128, 1024], F16, kind="ExternalInput")
    d_WrT = nc.dram_tensor("WrT", [128, 2048], F16, kind="ExternalInput")
    if has_bias:
        d_blstm = nc.dram_tensor("blstm", [128, 8], F32, kind="ExternalInput")
    d_H = nc.dram_tensor("Hout", [128, T * 32], F16, kind="ExternalOutput")
    d_c = nc.dram_tensor("cout", [128, 32], F32, kind="ExternalOutput")

    from contextlib import ExitStack
    ctx = ExitStack()
    sb_Xt = ctx.enter_context(nc.sbuf_tensor([128, T * BL], F16))
    sb_WkT = ctx.enter_context(nc.sbuf_tensor([128, 1024], F16))
    sb_WrT = ctx.enter_context(nc.sbuf_tensor([128, 2048], F16))
    if has_bias:
        sb_blstm = ctx.enter_context(nc.sbuf_tensor([128, 8], F32))
    sb_H = ctx.enter_context(nc.sbuf_tensor([128, (T + 1) * 32], F16))
    sb_G = ctx.enter_context(nc.sbuf_tensor([128, 2 * 128], F32))
    sb_TC = ctx.enter_context(nc.sbuf_tensor([128, 2 * 32], F32))
    sb_U = ctx.enter_context(nc.sbuf_tensor([128, 32], F32))
    sb_V = ctx.enter_context(nc.sbuf_tensor([128, 32], F32))
    sb_c = ctx.enter_context(nc.sbuf_tensor([128, 32], F32))

    ps_zg = [ctx.enter_context(nc.psum_tensor(f"ps_zg{j}", [128, 512], F32))
             for j in range(2)]
    ps_zif = [ctx.enter_context(nc.psum_tensor(f"ps_zif{j}", [128, 512], F32))
              for j in range(2)]
    ps_zo = [ctx.enter_context(nc.psum_tensor(f"ps_zo{j}", [128, 512], F32))
             for j in range(2)]
    ps_s = ctx.enter_context(nc.psum_tensor("ps_s", [128, 512], F32))

    dma_xt = ctx.enter_context(nc.semaphore("dma_xt"))
    dma_wk = ctx.enter_context(nc.semaphore("dma_wk"))
    dma_w0 = ctx.enter_context(nc.semaphore("dma_w0"))
    dma_w1 = ctx.enter_context(nc.semaphore("dma_w1"))
    if has_bias:
        dma_bl = ctx.enter_context(nc.semaphore("dma_bl"))
    dma_out = ctx.enter_context(nc.semaphore("dma_out"))
    sem_h = ctx.enter_context(nc.semaphore("sem_h"))
    sem_pe = ctx.enter_context(nc.semaphore("sem_pe"))
    sem_act = ctx.enter_context(nc.semaphore("sem_act"))
    sem_uv = ctx.enter_context(nc.semaphore("sem_uv"))
    sem_s = ctx.enter_context(nc.semaphore("sem_s"))
    sem_cv = ctx.enter_context(nc.semaphore("sem_cv"))

    # m-tile -> (bank pair, column offset, first-in-bank)
    def bank_of(m):
        if m < 2:
            return ps_zg, m * BL, m == 0
        if m < 6:
            return ps_zif, (m - 2) * BL, m == 2
        return ps_zo, (m - 6) * BL, m == 6

    with nc.Block() as block:

        @block.sync
        def _(sync):
            sync.dma_start(out=sb_WrT[:, 1024:2048], in_=d_WrT[:, 1024:2048]
                           ).then_inc(dma_w1, 16)
            if has_bias:
                sync.dma_start(out=sb_blstm[:], in_=d_blstm[:]
                               ).then_inc(dma_bl, 16)
            # whole h history in one DMA once the scan retires
            sync.wait_ge(sem_h, T + 1)
            sync.dma_start(out=d_H[:], in_=sb_H[:, 32:(T + 1) * 32]
                           ).then_inc(dma_out, 16)
            sync.wait_ge(dma_out, 32)

        @block.gpsimd
        def _(gpsimd):
            gpsimd.dma_start(out=sb_WrT[:, 0:1024], in_=d_WrT[:, 0:1024]
                             ).then_inc(dma_w0, 16)

        @block.tensor
        def _(tensor):
            tensor.wait_ge(dma_xt, 16)    # standalone: gates LDW stream too
            tensor.wait_ge(dma_wk, 16)
            tensor.wait_ge(dma_w0, 16)
            tensor.wait_ge(dma_w1, 16)

            for t in range(T):
                s2 = t % 2
                xt = sb_Xt[:, t * BL:(t + 1) * BL]
                # x-projection mms: no h dependency, run in the shadow of the
                # previous step's pointwise tail.  First mm into each bank
                # clears the whole bank's has_written bits (start=True); the
                # later ones write into cleared bits so they also overwrite.
                # Bank reuse is gated on step t-2's activation reads.
                for m in range(M_TILES):
                    bank, col, fst = bank_of(m)
                    mm = tensor.matmul(
                        bank[s2][:, col:col + BL],
                        sb_WkT[:, m * 128:(m + 1) * 128], xt,
                        start=fst, stop=False, skip_group_check=True)
                    if fst and t >= 2:
                        gate_idx = {0: 1, 2: 2, 6: 3}[m]
                        na = 4 if not has_bias else 4
                        mm.wait_op(sem_act, 4 * (t - 2) + gate_idx, "sem-ge")
                # recurrent matmuls; first carries the h(t) wait so the
                # LDWEIGHTS stream can prefetch past it
                first = True
                for m in range(M_TILES):
                    bank, col, _ = bank_of(m)
                    for k in range(K2):
                        mm = tensor.matmul(
                            bank[s2][:, col:col + BL],
                            sb_WrT[:, k * 1024 + m * 128:
                                   k * 1024 + (m + 1) * 128],
                            sb_H[:, t * 32 + k * BL:t * 32 + (k + 1) * BL],
                            start=False, stop=False, skip_group_check=True)
                        if first:
                            mm.wait_op(sem_h, t + 1, "sem-ge")
                            first = False
                    if m == 1 or m == 5 or m == 7:
                        mm.then_inc(sem_pe)   # g / i,f / o complete

        @block.scalar
        def _(scalar):
            Tanh = mybir.ActivationFunctionType.Tanh
            Sig = mybir.ActivationFunctionType.Sigmoid
            scalar.dma_start(out=sb_Xt[:], in_=d_Xt[:]).then_inc(dma_xt, 16)
            scalar.dma_start(out=sb_WkT[:], in_=d_WkT[:]).then_inc(dma_wk, 16)

            def act(dst, src, func, wait_val, inc, mslice=None):
                if mslice is None:
                    op = scalar.activation(dst, src, func)
                else:
                    op = scalar.activation(dst, src, func,
                                           bias=sb_blstm[:, mslice:mslice + 1])
                if wait_val is not None:
                    op.wait_op(sem_pe, wait_val, "sem-ge")
                if inc:
                    op.then_inc(sem_act)
                return op

            for t in range(T):
                s2 = t % 2
                gs = sb_G[:, s2 * 128:(s2 + 1) * 128]
                if not has_bias:
                    # A1 tanh(g): fires after 4 matmuls, under the PE stream
                    act(gs[:, 0:32], ps_zg[s2][:, 0:32], Tanh,
                        3 * t + 1, True)
                    act(gs[:, 32:96], ps_zif[s2][:, 0:64], Sig,
                        3 * t + 2, True)
                    act(gs[:, 96:128], ps_zo[s2][:, 0:32], Sig,
                        3 * t + 3, True)
                else:
                    # per-m activations so the per-gate-feature bias can ride
                    # the ACT bias port ([128,1] per 128-feature tile)
                    act(gs[:, 0:16], ps_zg[s2][:, 0:16], Tanh, 3 * t + 1,
                        False, 0)
                    act(gs[:, 16:32], ps_zg[s2][:, 16:32], Tanh, None,
                        True, 1)
                    act(gs[:, 32:48], ps_zif[s2][:, 0:16], Sig, 3 * t + 2,
                        False, 2)
                    act(gs[:, 48:64], ps_zif[s2][:, 16:32], Sig, None,
                        False, 3)
                    act(gs[:, 64:80], ps_zif[s2][:, 32:48], Sig, None,
                        False, 4)
                    act(gs[:, 80:96], ps_zif[s2][:, 48:64], Sig, None,
                        True, 5)
                    act(gs[:, 96:112], ps_zo[s2][:, 0:16], Sig, 3 * t + 3,
                        False, 6)
                    act(gs[:, 112:128], ps_zo[s2][:, 16:32], Sig, None,
                        True, 7)
                # A4: tanh(c')
                scalar.activation(sb_TC[:, s2 * 32:(s2 + 1) * 32],
                                  ps_s[:, s2 * 32:(s2 + 1) * 32], Tanh
                                  ).wait_op(sem_s, t + 1, "sem-ge"
                                            ).then_inc(sem_act)
            # final cell state DMA rides the (idle) scalar queue
            scalar.wait_ge(sem_cv, 1)
            scalar.dma_start(out=d_c[:], in_=sb_c[:]).then_inc(dma_out, 16)

        @block.vector
        def _(vector):
            vector.memset(sb_H[:, 0:32], 0.0).then_inc(sem_h)
            if has_bias:
                vector.wait_ge(dma_bl, 16)

            for t in range(T):
                s2 = t % 2
                gs = sb_G[:, s2 * 128:(s2 + 1) * 128]
                ss = ps_s[:, s2 * 32:(s2 + 1) * 32]
                cprev = ps_s[:, (1 - s2) * 32:(2 - s2) * 32]
                if t == 0:
                    # c0 = 0: c1 = i*g directly into psum
                    nc.vector.tensor_mul(
                        ss, gs[:, 32:64], gs[:, 0:32]
                    ).wait_op(sem_act, 4 * t + 2, "sem-ge").then_inc(sem_s)
                else:
                    nc.vector.tensor_mul(
                        sb_U[:], gs[:, 32:64], gs[:, 0:32]
                    ).wait_op(sem_act, 4 * t + 2, "sem-ge")
                    # v = f*c  (bank-reuse guard: A4(t-2) has read ps_s)
                    vop = nc.vector.tensor_mul(sb_V[:], gs[:, 64:96], cprev)
                    if t >= 2:
                        vop.wait_op(sem_act, 4 * (t - 2) + 4, "sem-ge")
                    vop.then_inc(sem_uv)
                    nc.vector.tensor_add(
                        ss, sb_U[:], sb_V[:]
                    ).wait_op(sem_uv, t, "sem-ge").then_inc(sem_s)
                # h = o * tanh(c')
                nc.vector.tensor_mul(
                    sb_H[:, (t + 1) * 32:(t + 2) * 32], gs[:, 96:128],
                    sb_TC[:, s2 * 32:(s2 + 1) * 32]
                ).wait_op(sem_act, 4 * t + 4, "sem-ge").then_inc(sem_h)
            # final cell state for the host fallback
            nc.vector.tensor_scalar_mul(
                sb_c[:], ps_s[:, ((T - 1) % 2) * 32:((T - 1) % 2 + 1) * 32],
                1.0).then_inc(sem_cv)

    return nc, ctx


_BUILD_CACHE = {}


def _get_nc(T, has_bias):
    key = (T, has_bias)
    if key not in _BUILD_CACHE:
        _BUILD_CACHE[key] = _build(T, has_bias)
    return _BUILD_CACHE[key][0]


def _prep_inputs(X, Wk, Wr, b_lstm, T, has_bias):
    """Build the 8 per-core input maps (numpy, host-side sharding)."""
    Wk_p = np.ascontiguousarray(Wk[:, GATE_PERM]).astype(np.float16)
    Wr_p = Wr[:, GATE_PERM].astype(np.float32)
    WrT = np.ascontiguousarray(
        Wr_p.reshape(2, 128, 1024).transpose(1, 0, 2).reshape(128, 2048)
    ).astype(np.float16)
    base = {"WkT": Wk_p, "WrT": WrT}
    if has_bias:
        base["blstm"] = np.ascontiguousarray(
            b_lstm[GATE_PERM].astype(np.float32).reshape(8, 128).T)
    in_maps = []
    for i in range(NCORES):
        bsl = slice(i * BL, (i + 1) * BL)
        Xt = np.ascontiguousarray(
            X[bsl, :T, :].astype(np.float32).transpose(2, 1, 0)
            .reshape(128, T * BL)).astype(np.float16)
        m = dict(base)
        m["Xt"] = Xt
        in_maps.append(m)
    return in_maps


def _sigmoid64(x):
    return 1.0 / (1.0 + np.exp(-x.astype(np.float64)))


def _softmax32(x):
    x = x.astype(np.float32)
    e = np.exp(x - x.max(axis=-1, keepdims=True))
    return (e / e.sum(axis=-1, keepdims=True)).astype(np.float32)


def _fallback_scan(x_seq, u_seq, h0, c0, t0, Wk, Wr, b_lstm, Wo, bo, Wc, bc):
    """Continue the reference recurrence on host for one sample that did not
    halt by t0.  Returns the sample's output row (float32)."""
    h = h0.astype(np.float32).copy()
    c = c0.astype(np.float32).copy()
    Wk = Wk.astype(np.float32); Wr = Wr.astype(np.float32)
    b_lstm = b_lstm.astype(np.float32)
    sig = lambda v: 1.0 / (1.0 + np.exp(-v))
    Tt = x_seq.shape[0]
    logits_last = None
    for t in range(t0, Tt):
        z = x_seq[t] @ Wk + h @ Wr + b_lstm
        i, f, g, o = np.split(z, 4)
        i = sig(i); f = sig(f); g = np.tanh(g); o = sig(o)
        c = f * c + i * g
        h = o * np.tanh(c)
        y = h @ Wo.astype(np.float32) + bo.astype(np.float32)
        logits = _softmax32(y)
        pre = float(h @ Wc[:256, 0].astype(np.float32)) \
            + t * float(Wc[256, 0]) + float(bc[0])
        probs = (1.0 - EPS) * sig(np.float32(pre)) + EPS * 0.05
        if u_seq[t] < probs:
            return logits
        logits_last = logits
    return logits_last


def kernel(**inputs):
    X = np.asarray(inputs["X"], np.float32)
    u = np.asarray(inputs["u"], np.float32)
    Wk = np.asarray(inputs["Wk"], np.float32)
    Wr = np.asarray(inputs["Wr"], np.float32)
    b_lstm = np.asarray(inputs["b_lstm"], np.float32)
    Wo = np.asarray(inputs["Wo"], np.float32)
    bo = np.asarray(inputs["bo"], np.float32)
    Wc = np.asarray(inputs["Wc"], np.float32)
    bc = np.asarray(inputs["bc"], np.float32)
    T = T_EFF
    has_bias = bool(np.any(b_lstm))

    nc = _get_nc(T, has_bias)
    in_maps = _prep_inputs(X, Wk, Wr, b_lstm, T, has_bias)
    res = run_bass_kernel_spmd(nc, in_maps, list(range(NCORES)))

    wc_t = float(Wc[256, 0])
    bias_c = float(bc[0])
    tvec = np.arange(T, dtype=np.float64)
    Wo64 = Wo.astype(np.float64)
    Wc64 = Wc[:256, 0].astype(np.float64)

    out = np.zeros((B, C), np.float32)
    for i in range(NCORES):
        bsl = slice(i * BL, (i + 1) * BL)
        hraw = res.results[i]["Hout"]         # [128, T*32] fp16
        # cols: t*32 + k*16 + b ; partitions: feature within k-tile
        h_hist = hraw.reshape(128, T, 2, BL).transpose(1, 3, 2, 0) \
            .reshape(T, BL, 256).astype(np.float64)   # h after step t
        y = h_hist @ Wo64 + bo.astype(np.float64)     # [T, b, C]
        pre_c = h_hist @ Wc64 + tvec[:, None] * wc_t + bias_c  # [T, b]
        probs = (1.0 - EPS) * _sigmoid64(pre_c) + EPS * 0.05
        u_core = u[bsl, :T, 0]                 # [b, T]
        a = u_core.T.astype(np.float64) < probs  # [T, b]
        halted = a.any(axis=0)
        tstar = np.argmax(a, axis=0)
        logits = _softmax32(y)                 # [T, b, C]
        craw = res.results[i]["cout"]          # [128, 32] fp32
        c_T = craw.reshape(128, 2, BL).transpose(2, 1, 0).reshape(BL, 256)
        for b_ in range(BL):
            if halted[b_]:
                out[i * BL + b_] = logits[tstar[b_], b_]
            else:
                out[i * BL + b_] = _fallback_scan(
                    X[i * BL + b_], u[i * BL + b_, :, 0],
                    h_hist[T - 1, b_].astype(np.float32), c_T[b_], T,
                    Wk, Wr, b_lstm, Wo, bo, Wc, bc)
    return out
